# revision 1
# baseline (speedup 1.0000x reference)
"""Causal self-attention (B=4, T=2048, C=1024, 16 heads) on 8 trn2 NeuronCores.

Sharding: tensor-parallel over heads (2 heads/core) for QKV + attention,
then an AllToAll reshards from head-split to token-split for the output
projection.  Each core returns out[token_slice, :]; the host concatenates.

Per-core pipeline (all cores run the identical program; only the fed
W_qkv column-slice differs):
  stage 1: qT,kT  [128ch x 2048tok]  and v [tok-major] per batch, from
           x @ W_qkv_slice  (x is DMA'd in transposed [c, t] tiles)
  stage 2: causal attention per (batch, head): S^T tiles [kt=128, q=512],
           exp on ACT (no max-subtraction: scores/8 ~ N(0,1), bounded),
           multiplicative 0/1 causal mask on diagonal blocks,
           AV accumulation with a ones-column appended to v so PSUM row 64
           carries the softmax denominators; normalize via reciprocal +
           partition_broadcast.
  stage 3: AllToAll (4.2MB/rank) -> y^T [1024ch, 1024tok] token slice,
           out = y^T.T @ W_proj  accumulated over 8 channel chunks.
"""

import os
import numpy as np

from concourse import bass, bacc, mybir, tile
from concourse.bass_utils import run_bass_kernel_spmd

F32 = mybir.dt.float32
F32R = mybir.dt.float32r
BF16 = mybir.dt.bfloat16

B, T, C = 4, 2048, 1024
H, D = 16, 64
NCORES = 8
HPC = H // NCORES            # heads per core = 2
QKC = HPC * D                # per-core q/k/v channels = 128
BT = B * T                   # 8192 tokens total
TPS = BT // NCORES           # tokens per core after A2A = 1024
P = 128
TW = 512                     # token window for stage1/attention q windows
NW = T // TW                 # windows per batch = 4
NKT = T // P                 # kt tiles per batch = 16

# ---- dtype knobs (bitcast matmul operands; float32r = full-rate PE) ----
RD = {"f32": F32, "f32r": F32R, "bf16": BF16}[os.environ.get("KMM_DT", "f32")]
PSS_BUFS = int(os.environ.get("KPSS", "4"))
PSY_BUFS = int(os.environ.get("KPSY", "2"))
DIAG_FIRST = os.environ.get("KDIAG", "1") == "1"
YU_COPY = os.environ.get("KYU", "1") == "1"


def _mm(ap, dt=None):
    return ap


def _causal_mask_01() -> np.ndarray:
    """mask[p, m, f] = 1.0 iff kt_local = 128*m + p <= f, for q windows of 512."""
    m = np.zeros((P, NW, TW), dtype=np.float32)
    p = np.arange(P)[:, None, None]
    mm = np.arange(NW)[None, :, None]
    f = np.arange(TW)[None, None, :]
    m[(P * mm + p) <= f] = 1.0
    return m


def build() -> bass.Bass:
    nc = bacc.Bacc(num_devices=NCORES, target_bir_lowering=False)

    x_d = nc.dram_tensor("x", [BT, C], F32, kind="ExternalInput")
    wqkv_d = nc.dram_tensor("wqkv", [C, 3 * QKC], F32, kind="ExternalInput")
    wproj_d = nc.dram_tensor("wproj", [C, C], F32, kind="ExternalInput")
    out_d = nc.dram_tensor("out", [TPS, C], F32, kind="ExternalOutput")

    mask_d = nc.inline_tensor(_causal_mask_01(), name="mask01")
    ident_d = nc.inline_tensor(np.eye(P, dtype=np.float32), name="ident")

    KC = C // P  # 8 contraction chunks

    with tile.TileContext(nc) as tc:
        from contextlib import ExitStack

        with ExitStack() as ctx:
            # ---- persistent pools ----
            wq_pool = ctx.enter_context(tc.tile_pool(name="wq", bufs=1))
            msk_pool = ctx.enter_context(tc.tile_pool(name="msk", bufs=1))
            wp_pool = ctx.enter_context(tc.tile_pool(name="wp", bufs=1))
            dram = ctx.enter_context(tc.tile_pool(name="dram", bufs=1, space="DRAM"))

            wqkv_sb = wq_pool.tile([P, KC, 3 * QKC], RD)
            mask_sb = msk_pool.tile([P, NW, TW], RD)
            ident_sb = msk_pool.tile([P, P], F32)
            nc.sync.dma_start(out=ident_sb[:], in_=ident_d[:, :])
            if RD is F32:
                ident_rd = ident_sb
            else:
                ident_rd = msk_pool.tile([P, P], RD)
                nc.vector.tensor_copy(ident_rd[:], ident_sb[:])
            wproj_sb = wp_pool.tile([P, KC, C], RD)
            if RD is F32:
                nc.sync.dma_start(
                    out=wqkv_sb[:], in_=x_dram_re(wqkv_d, "(k p) n -> p k n")
                )
                nc.sync.dma_start(out=mask_sb[:], in_=mask_d[:, :, :])
                nc.sync.dma_start(
                    out=wproj_sb[:], in_=x_dram_re(wproj_d, "(k p) n -> p k n")
                )
            else:
                with tc.tile_pool(name="stage", bufs=2) as stg:
                    wqkv_st = stg.tile([P, KC, 3 * QKC], F32, tag="st3", bufs=1)
                    nc.sync.dma_start(
                        out=wqkv_st[:], in_=x_dram_re(wqkv_d, "(k p) n -> p k n")
                    )
                    nc.vector.tensor_copy(wqkv_sb[:], wqkv_st[:])
                    mask_st = stg.tile([P, NW, TW], F32, tag="stm", bufs=1)
                    nc.sync.dma_start(out=mask_st[:], in_=mask_d[:, :, :])
                    nc.vector.tensor_copy(mask_sb[:], mask_st[:])
                    for kc in range(KC):
                        wproj_st = stg.tile([P, C], F32, tag="wst", name="wproj_st")
                        nc.sync.dma_start(
                            out=wproj_st[:],
                            in_=wproj_d[kc * P : (kc + 1) * P, :],
                        )
                        nc.vector.tensor_copy(wproj_sb[:, kc, :], wproj_st[:])

            y_send = dram.tile([NCORES, QKC, TPS], RD)
            y_recv = dram.tile([NCORES, QKC, TPS], RD)

            # ---- stage 1 + 2 pools ----
            s12 = ExitStack()
            with s12:
                xT_pool = s12.enter_context(tc.tile_pool(name="xT", bufs=2))
                qkv_pool = s12.enter_context(tc.tile_pool(name="qkv", bufs=2))
                ps1 = s12.enter_context(
                    tc.tile_pool(name="ps1", bufs=2, space="PSUM")
                )
                pss = s12.enter_context(
                    tc.tile_pool(name="pss", bufs=PSS_BUFS, space="PSUM")
                )
                psy = s12.enter_context(
                    tc.tile_pool(name="psy", bufs=PSY_BUFS, space="PSUM")
                )
                pt_pool = s12.enter_context(tc.tile_pool(name="pt", bufs=6))
                nrm_pool = s12.enter_context(tc.tile_pool(name="nrm", bufs=2))
                yt_pool = s12.enter_context(tc.tile_pool(name="yt", bufs=2))

                for b in range(B):
                    qT_b = qkv_pool.tile([P, T], RD, tag="qT")
                    kT_b = qkv_pool.tile([P, T], RD, tag="kT")
                    v_b = qkv_pool.tile([P, NKT, HPC, D + 1], RD, tag="v")
                    # ones column for softmax denominators
                    ones_col = v_b[:, :, :, D : D + 1]
                    if RD is F32R:
                        ones_col = ones_col.bitcast(F32)
                    nc.gpsimd.memset(ones_col, 1.0)

                    # ---- stage 1: qT, kT, v for batch b ----
                    for w in range(NW):
                        t0 = b * T + w * TW
                        # natural-layout x subtiles (contiguous 4KB-row DMAs)
                        xns = []
                        for s in range(TW // P):
                            xn = xT_pool.tile([P, C], F32, tag="xn", name="xn", bufs=6)
                            nc.sync.dma_start(
                                out=xn[:], in_=x_d[t0 + s * P : t0 + (s + 1) * P, :]
                            )
                            xns.append(xn)
                        # transpose to xT [c-part, tok] on the PE
                        xT = xT_pool.tile([P, KC, TW], RD)
                        for kc in range(KC):
                            ps_t = ps1.tile([P, TW], F32, tag="ps1", name="ps_t")
                            for s in range(TW // P):
                                nc.tensor.transpose(
                                    ps_t[:, s * P : (s + 1) * P],
                                    xns[s][:, kc * P : (kc + 1) * P],
                                    ident_sb[:],
                                )
                            nc.vector.tensor_copy(xT[:, kc, :], ps_t[:])
                        for which, dst in ((0, qT_b), (1, kT_b)):
                            ps = ps1.tile([P, TW], F32, tag="ps1")
                            for kc in range(KC):
                                nc.tensor.matmul(
                                    ps[:],
                                    lhsT=wqkv_sb[:, kc, which * QKC : (which + 1) * QKC],
                                    rhs=xT[:, kc, :],
                                    start=(kc == 0),
                                    stop=(kc == KC - 1),
                                )
                            nc.vector.tensor_copy(dst[:, w * TW : (w + 1) * TW], ps[:])
                        ps_vT = ps1.tile([P, TW], F32, tag="ps1", name="ps_vT")
                        for kc in range(KC):
                            nc.tensor.matmul(
                                ps_vT[:],
                                lhsT=wqkv_sb[:, kc, 2 * QKC : 3 * QKC],
                                rhs=xT[:, kc, :],
                                start=(kc == 0),
                                stop=(kc == KC - 1),
                            )
                        vT_sb = xT_pool.tile([P, TW], RD, tag="vT", name="vT_sb")
                        nc.vector.tensor_copy(vT_sb[:], ps_vT[:])
                        ps_v = ps1.tile([P, TW], RD, tag="ps1", name="ps_v")
                        for s in range(TW // P):
                            nc.tensor.transpose(
                                ps_v[:, s * P : (s + 1) * P],
                                vT_sb[:, s * P : (s + 1) * P],
                                ident_rd[:],
                            )
                        jt0 = w * (TW // P)
                        nc.vector.tensor_copy(
                            v_b[:, jt0 : jt0 + TW // P, :, 0:D],
                            ps_v[:].rearrange("p (s h d) -> p s h d", s=TW // P, h=HPC),
                        )

                    # ---- stage 2: attention for batch b ----
                    # h innermost: two independent AV chains overlap on PE
                    for w in range(NW):
                        for h in range(HPC):
                            qT_h = qT_b[h * D : (h + 1) * D, :]
                            kT_h = kT_b[h * D : (h + 1) * D, :]
                            nkt = (w + 1) * (TW // P)
                            ps_y = psy.tile([D + 1, TW], F32, tag="ps_y")
                            jks = list(range(nkt))
                            if DIAG_FIRST:
                                jks = jks[w * (TW // P):] + jks[: w * (TW // P)]
                            for ji, jk in enumerate(jks):
                                ps_s = pss.tile([P, TW], F32, tag="ps_s")
                                nc.tensor.matmul(
                                    ps_s[:],
                                    lhsT=kT_h[:, jk * P : (jk + 1) * P],
                                    rhs=qT_h[:, w * TW : (w + 1) * TW],
                                    start=True,
                                    stop=True,
                                )
                                pt = pt_pool.tile([P, TW], RD, tag="pt")
                                nc.scalar.activation(
                                    pt[:],
                                    ps_s[:],
                                    mybir.ActivationFunctionType.Exp,
                                    scale=1.0 / np.sqrt(D),
                                )
                                m = jk - w * (TW // P)
                                if m >= 0:
                                    nc.gpsimd.tensor_mul(
                                        pt[:], pt[:], mask_sb[:, m, :]
                                    )
                                nc.tensor.matmul(
                                    ps_y[:],
                                    lhsT=v_b[:, jk, h, :],
                                    rhs=pt[:],
                                    start=(ji == 0),
                                    stop=(ji == nkt - 1),
                                )
                            if YU_COPY:
                                yu = yt_pool.tile([D + 1, TW], F32, tag="yu", bufs=4)
                                nc.vector.tensor_copy(yu[:], ps_y[:])
                                src_y = yu
                            else:
                                src_y = ps_y
                            recip = nrm_pool.tile([1, TW], F32, tag="recip")
                            nc.vector.reciprocal(recip[:], src_y[D : D + 1, :])
                            bc = nrm_pool.tile([D, TW], F32, tag="bc")
                            nc.gpsimd.partition_broadcast(bc[:], recip[:])
                            yt = yt_pool.tile([D, TW], RD, tag="yt")
                            nc.vector.tensor_mul(yt[:], src_y[0:D, :], bc[:])
                            g0 = b * T + w * TW
                            shard = g0 // TPS
                            c0 = g0 % TPS
                            nc.sync.dma_start(
                                out=y_send[shard, h * D : (h + 1) * D, c0 : c0 + TW],
                                in_=yt[:],
                            )

            # ---- stage 3: A2A + projection ----
            nc.gpsimd.collective_compute(
                "AllToAll",
                mybir.AluOpType.bypass,
                replica_groups=[list(range(NCORES))],
                ins=[y_send.opt()],
                outs=[y_recv.opt()],
            )

            s3 = ExitStack()
            with s3:
                yr_pool = s3.enter_context(tc.tile_pool(name="yr", bufs=2))
                pso = s3.enter_context(tc.tile_pool(name="pso", bufs=2, space="PSUM"))
                ob_pool = s3.enter_context(tc.tile_pool(name="ob", bufs=2))
                for jt in range(TPS // P):
                    yr = yr_pool.tile([P, KC, P], RD, tag="yr")
                    nc.sync.dma_start(
                        out=yr[:],
                        in_=y_recv[:, :, jt * P : (jt + 1) * P].rearrange(
                            "k p t -> p k t"
                        ),
                    )
                    for half in range(C // TW):
                        ps_o = pso.tile([P, TW], F32, tag="ps_o")
                        for kc in range(KC):
                            nc.tensor.matmul(
                                ps_o[:],
                                lhsT=yr[:, kc, :],
                                rhs=wproj_sb[:, kc, half * TW : (half + 1) * TW],
                                start=(kc == 0),
                                stop=(kc == KC - 1),
                            )
                        ob = ob_pool.tile([P, TW], F32, tag="ob")
                        nc.vector.tensor_copy(ob[:], ps_o[:])
                        nc.sync.dma_start(
                            out=out_d[jt * P : (jt + 1) * P, half * TW : (half + 1) * TW],
                            in_=ob[:],
                        )

    nc.finalize()
    return nc


def x_dram_re(handle, pattern):
    return handle[:, :].rearrange(pattern, p=P)


_NC_CACHE: dict = {}


def _get_nc() -> bass.Bass:
    if "nc" not in _NC_CACHE:
        _NC_CACHE["nc"] = build()
    return _NC_CACHE["nc"]


def shard_inputs(x, W_qkv, W_proj):
    x = np.ascontiguousarray(np.asarray(x, dtype=np.float32).reshape(BT, C))
    W_qkv = np.asarray(W_qkv, dtype=np.float32)
    W_proj = np.ascontiguousarray(np.asarray(W_proj, dtype=np.float32))
    in_maps = []
    for c in range(NCORES):
        cols = slice(QKC * c, QKC * (c + 1))
        w_c = np.ascontiguousarray(
            np.concatenate(
                [W_qkv[:, cols], W_qkv[:, C:][:, cols], W_qkv[:, 2 * C :][:, cols]],
                axis=1,
            )
        )
        in_maps.append({"x": x, "wqkv": w_c, "wproj": W_proj})
    return in_maps


def run(in_maps, trace=False, **kwargs):
    return run_bass_kernel_spmd(
        _get_nc(), in_maps, core_ids=list(range(NCORES)), trace=trace, **kwargs
    )


def kernel(x, W_qkv, W_proj):
    res = run(shard_inputs(x, W_qkv, W_proj), trace=False)
    out = np.concatenate([res.results[c]["out"] for c in range(NCORES)], axis=0)
    return out.reshape(B, T, C).astype(np.float32)



# revision 15
# speedup vs baseline: 1.3551x; 1.3551x over previous
"""Causal self-attention (B=4, T=2048, C=1024, 16 heads) on 8 trn2 NeuronCores.

Sharding: tensor-parallel over heads (2 heads/core) for QKV + attention.
Output tokens are interleaved across cores: core c owns, for every batch b,
the 256-token slice [b*2048 + c*256, b*2048 + (c+1)*256).  This lets the
head-split -> token-split reshard run as FOUR small per-batch AllToAlls
(512KB each), each issued right after its batch's attention finishes and
hidden under the next batch's compute; the output projection for batch b
runs inside the main loop as soon as A2A_b lands.

Per-core pipeline (all cores run the identical program; only the fed
W_qkv column-slice differs):
  stage 1: qT,kT  [128ch x 2048tok]  and v [tok-major] per batch, from
           x @ W_qkv_slice.  x tiles are pre-cast to bf16 on the Scalar
           engine so the PE transposes run at 1 cycle/row.
  stage 2: causal attention per (batch, head): S^T tiles [kt=128, q<=512],
           software-pipelined with LOOKAHEAD S-matmuls in flight ahead of
           the AV accumulation so the PE never waits on exp/mask.
           Diagonal tiles skip their fully-masked leading columns.
           exp on ACT (no max-subtraction: scores/8 ~ N(0,1), bounded),
           ones-column appended to v so PSUM row 64 carries the softmax
           denominators; normalize via reciprocal_approx_fast +
           partition_broadcast.
  stage 3 (per batch): A2A_b -> y^T [1024ch, 256tok] token slice,
           out = y^T.T @ W_proj, interleaved after the next batch's
           stage 1 so the collective latency is hidden.
"""

import os
import numpy as np

from concourse import bass, bacc, mybir, tile
from concourse.bass_utils import run_bass_kernel_spmd

F32 = mybir.dt.float32
F32R = mybir.dt.float32r
BF16 = mybir.dt.bfloat16

B, T, C = 4, 2048, 1024
H, D = 16, 64
NCORES = 8
HPC = H // NCORES            # heads per core = 2
QKC = HPC * D                # per-core q/k/v channels = 128
BT = B * T                   # 8192 tokens total
TPS = BT // NCORES           # tokens per core in the output = 1024
TB = TPS // B                # tokens per (core, batch) out slice = 256
P = 128
TW = 512                     # token window for stage1/attention q windows
NW = T // TW                 # windows per batch = 4
NKT = T // P                 # kt tiles per batch = 16

# ---- knobs ----
RD = {"f32": F32, "f32r": F32R, "bf16": BF16}[os.environ.get("KMM_DT", "bf16")]
PSS_BUFS = int(os.environ.get("KPSS", "4"))
PSY_BUFS = int(os.environ.get("KPSY", "2"))
LOOKAHEAD = int(os.environ.get("KLA", "3"))
XCAST = os.environ.get("KXCAST", "1") == "1" and RD is not F32
DBG = os.environ.get("KDBG", "0") == "1"


def _causal_mask_01() -> np.ndarray:
    """mask[p, m, f] = 1.0 iff kt_local = 128*m + p <= f, for q windows of 512."""
    m = np.zeros((P, NW, TW), dtype=np.float32)
    p = np.arange(P)[:, None, None]
    mm = np.arange(NW)[None, :, None]
    f = np.arange(TW)[None, None, :]
    m[(P * mm + p) <= f] = 1.0
    return m


def build() -> bass.Bass:
    nc = bacc.Bacc(num_devices=NCORES, target_bir_lowering=False)

    x_d = nc.dram_tensor("x", [BT, C], F32, kind="ExternalInput")
    wqkv_d = nc.dram_tensor("wqkv", [C, 3 * QKC], F32, kind="ExternalInput")
    wproj_d = nc.dram_tensor("wproj", [C, C], F32, kind="ExternalInput")
    out_d = nc.dram_tensor("out", [TPS, C], F32, kind="ExternalOutput")

    mask_d = nc.inline_tensor(_causal_mask_01(), name="mask01")
    ident_d = nc.inline_tensor(np.eye(P, dtype=np.float32), name="ident")

    if DBG:
        dbgq_d = nc.dram_tensor("dbg_q", [P, T], RD, kind="ExternalOutput")
        dbgden_d = nc.dram_tensor("dbg_den", [NW * HPC, TW], F32, kind="ExternalOutput")
        dbgrcp_d = nc.dram_tensor("dbg_rcp", [NW * HPC, TW], F32, kind="ExternalOutput")
        dbgy_d = nc.dram_tensor("dbg_y", [NW * HPC, D, TW], RD, kind="ExternalOutput")
        dbgyr_d = nc.dram_tensor("dbg_yr", [P, C // P, TB], RD, kind="ExternalOutput")

    KC = C // P  # 8 contraction chunks

    with tile.TileContext(nc) as tc:
        from contextlib import ExitStack

        with ExitStack() as ctx:
            # ---- persistent pools ----
            wq_pool = ctx.enter_context(tc.tile_pool(name="wq", bufs=1))
            msk_pool = ctx.enter_context(tc.tile_pool(name="msk", bufs=1))
            wp_pool = ctx.enter_context(tc.tile_pool(name="wp", bufs=1))
            stg_pool = ctx.enter_context(tc.tile_pool(name="stg", bufs=2))
            dram = ctx.enter_context(tc.tile_pool(name="dram", bufs=1, space="DRAM"))

            wqkv_sb = wq_pool.tile([P, KC, 3 * QKC], RD)
            mask_sb = msk_pool.tile([P, NW, TW], RD)
            ident_sb = msk_pool.tile([P, P], F32)
            nc.sync.dma_start(out=ident_sb[:], in_=ident_d[:, :])
            if RD is F32:
                ident_rd = ident_sb
            else:
                ident_rd = msk_pool.tile([P, P], RD)
                nc.vector.tensor_copy(ident_rd[:], ident_sb[:])
            wproj_sb = wp_pool.tile([P, KC, C], RD)
            if RD is F32:
                nc.sync.dma_start(
                    out=wqkv_sb[:], in_=x_dram_re(wqkv_d, "(k p) n -> p k n")
                )
                nc.sync.dma_start(out=mask_sb[:], in_=mask_d[:, :, :])
            else:
                wqkv_st = stg_pool.tile([P, KC, 3 * QKC], F32, tag="st3", bufs=1)
                nc.sync.dma_start(
                    out=wqkv_st[:], in_=x_dram_re(wqkv_d, "(k p) n -> p k n")
                )
                nc.vector.tensor_copy(wqkv_sb[:], wqkv_st[:])
                mask_st = stg_pool.tile([P, NW, TW], F32, tag="stm", bufs=1)
                nc.sync.dma_start(out=mask_st[:], in_=mask_d[:, :, :])
                nc.vector.tensor_copy(mask_sb[:], mask_st[:])

            def load_wproj():
                # issued after batch 0's stage 1 so it doesn't steal DMA
                # bandwidth from the startup-critical x/wqkv loads
                if RD is F32:
                    nc.sync.dma_start(
                        out=wproj_sb[:], in_=x_dram_re(wproj_d, "(k p) n -> p k n")
                    )
                else:
                    for kc in range(KC):
                        wproj_st = stg_pool.tile([P, C], F32, tag="wst")
                        nc.sync.dma_start(
                            out=wproj_st[:],
                            in_=wproj_d[kc * P : (kc + 1) * P, :],
                        )
                        nc.vector.tensor_copy(wproj_sb[:, kc, :], wproj_st[:])

            y_send = dram.tile([B, NCORES, QKC, TB], RD)
            y_recv = dram.tile([B, NCORES, QKC, TB], RD)

            # ---- stage pools (persistent: stages interleave across batches) ----
            xT_pool = ctx.enter_context(tc.tile_pool(name="xT", bufs=2))
            qkv_pool = ctx.enter_context(tc.tile_pool(name="qkv", bufs=2))
            ps1 = ctx.enter_context(tc.tile_pool(name="ps1", bufs=2, space="PSUM"))
            pss = ctx.enter_context(
                tc.tile_pool(name="pss", bufs=PSS_BUFS, space="PSUM")
            )
            psy = ctx.enter_context(
                tc.tile_pool(name="psy", bufs=PSY_BUFS, space="PSUM")
            )
            pt_pool = ctx.enter_context(tc.tile_pool(name="pt", bufs=6))
            nrm_pool = ctx.enter_context(tc.tile_pool(name="nrm", bufs=2))
            yt_pool = ctx.enter_context(tc.tile_pool(name="yt", bufs=4))
            yr_pool = ctx.enter_context(tc.tile_pool(name="yr", bufs=2))
            ob_pool = ctx.enter_context(tc.tile_pool(name="ob", bufs=2))

            def stage1(b):
                """qT, kT (ch-major) and v (tok-major) for batch b."""
                qT_b = qkv_pool.tile([P, T], RD, tag="qT")
                kT_b = qkv_pool.tile([P, T], RD, tag="kT")
                v_b = qkv_pool.tile([P, NKT, HPC, D + 1], RD, tag="v")
                # ones column for softmax denominators
                ones_col = v_b[:, :, :, D : D + 1]
                if RD is F32R:
                    ones_col = ones_col.bitcast(F32)
                nc.gpsimd.memset(ones_col, 1.0)

                for w in range(NW):
                    t0 = b * T + w * TW
                    # natural-layout x subtiles (contiguous 4KB-row DMAs)
                    xrs = []
                    for s in range(TW // P):
                        xn = xT_pool.tile([P, C], F32, tag="xn", name="xn", bufs=6)
                        nc.sync.dma_start(
                            out=xn[:], in_=x_d[t0 + s * P : t0 + (s + 1) * P, :]
                        )
                        if XCAST and RD is BF16:
                            # cast on the Scalar engine (idle during stage 1)
                            # so the PE transposes run at 1 cycle/row
                            xb = xT_pool.tile([P, C], RD, tag="xb", bufs=6)
                            nc.scalar.copy(xb[:], xn[:])
                            xrs.append(xb)
                        elif XCAST and RD is F32R:
                            xrs.append(xn[:].bitcast(F32R))
                        else:
                            xrs.append(xn)
                    idm = ident_rd if XCAST else ident_sb
                    ps_dt = RD if XCAST else F32
                    # transpose to xT [c-part, tok] on the PE
                    xT = xT_pool.tile([P, KC, TW], RD)
                    for kc in range(KC):
                        ps_t = ps1.tile([P, TW], ps_dt, tag="ps1", name="ps_t")
                        for s in range(TW // P):
                            nc.tensor.transpose(
                                ps_t[:, s * P : (s + 1) * P],
                                xrs[s][:, kc * P : (kc + 1) * P],
                                idm[:],
                            )
                        nc.vector.tensor_copy(xT[:, kc, :], ps_t[:])
                    for which, dst in ((0, qT_b), (1, kT_b)):
                        ps = ps1.tile([P, TW], F32, tag="ps1")
                        for kc in range(KC):
                            nc.tensor.matmul(
                                ps[:],
                                lhsT=wqkv_sb[:, kc, which * QKC : (which + 1) * QKC],
                                rhs=xT[:, kc, :],
                                start=(kc == 0),
                                stop=(kc == KC - 1),
                            )
                        nc.vector.tensor_copy(dst[:, w * TW : (w + 1) * TW], ps[:])
                    ps_vT = ps1.tile([P, TW], F32, tag="ps1", name="ps_vT")
                    for kc in range(KC):
                        nc.tensor.matmul(
                            ps_vT[:],
                            lhsT=wqkv_sb[:, kc, 2 * QKC : 3 * QKC],
                            rhs=xT[:, kc, :],
                            start=(kc == 0),
                            stop=(kc == KC - 1),
                        )
                    vT_sb = xT_pool.tile([P, TW], RD, tag="vT", name="vT_sb")
                    nc.vector.tensor_copy(vT_sb[:], ps_vT[:])
                    ps_v = ps1.tile([P, TW], RD, tag="ps1", name="ps_v")
                    for s in range(TW // P):
                        nc.tensor.transpose(
                            ps_v[:, s * P : (s + 1) * P],
                            vT_sb[:, s * P : (s + 1) * P],
                            ident_rd[:],
                        )
                    jt0 = w * (TW // P)
                    nc.vector.tensor_copy(
                        v_b[:, jt0 : jt0 + TW // P, :, 0:D],
                        ps_v[:].rearrange("p (s h d) -> p s h d", s=TW // P, h=HPC),
                    )
                return qT_b, kT_b, v_b

            def chains(b, qkv_tiles):
                """Attention chains for batch b, then the per-batch A2A."""
                qT_b, kT_b, v_b = qkv_tiles
                for w in range(NW):
                    for h in range(HPC):
                        qT_h = qT_b[h * D : (h + 1) * D, :]
                        kT_h = kT_b[h * D : (h + 1) * D, :]
                        nkt = (w + 1) * (TW // P)
                        # (jk, col_start): diagonal tiles first; diag tile s
                        # skips its fully-masked first 128*s columns.  The
                        # last item in issue order must be full-width so the
                        # PSUM accumulation group closes over all columns.
                        items = []
                        for s in range(TW // P):
                            cs = P * s
                            if w == 0 and s == TW // P - 1:
                                cs = 0
                            items.append((w * (TW // P) + s, cs))
                        items += [(jk, 0) for jk in range(w * (TW // P))]
                        pts = []

                        def issue_s(ji):
                            jk, cs = items[ji]
                            m = jk - w * (TW // P)
                            ps_s = pss.tile([P, TW], F32, tag="ps_s")
                            nc.tensor.matmul(
                                ps_s[:, cs:],
                                lhsT=kT_h[:, jk * P : (jk + 1) * P],
                                rhs=qT_h[:, w * TW + cs : (w + 1) * TW],
                                start=True,
                                stop=True,
                            )
                            pt = pt_pool.tile([P, TW], RD, tag="pt")
                            nc.scalar.activation(
                                pt[:, cs:],
                                ps_s[:, cs:],
                                mybir.ActivationFunctionType.Exp,
                                scale=1.0 / np.sqrt(D),
                            )
                            if m >= 0:
                                nc.gpsimd.tensor_mul(
                                    pt[:, cs:], pt[:, cs:], mask_sb[:, m, cs:]
                                )
                            pts.append(pt)

                        for ji in range(min(LOOKAHEAD, nkt)):
                            issue_s(ji)
                        ps_y = psy.tile([D + 1, TW], F32, tag="ps_y")
                        for ji in range(nkt):
                            if ji + LOOKAHEAD < nkt:
                                issue_s(ji + LOOKAHEAD)
                            jk, cs = items[ji]
                            nc.tensor.matmul(
                                ps_y[:, cs:],
                                lhsT=v_b[:, jk, h, :],
                                rhs=pts[ji][:, cs:],
                                start=(ji == 0),
                                stop=(ji == nkt - 1),
                            )
                        yu = yt_pool.tile([D + 1, TW], F32, tag="yu", bufs=4)
                        nc.vector.tensor_copy(yu[:], ps_y[:])
                        # reciprocal_approx_fast misreads partition-offset
                        # inputs; stage the denominator row at partition 0
                        den0 = nrm_pool.tile([1, TW], F32, tag="den0")
                        nc.vector.tensor_copy(den0[:], yu[D : D + 1, :])
                        recip = nrm_pool.tile([1, TW], F32, tag="recip")
                        nc.vector.reciprocal_approx_fast(recip[:], den0[:])
                        bc = nrm_pool.tile([D, TW], F32, tag="bc")
                        nc.gpsimd.partition_broadcast(bc[:], recip[:])
                        yt = yt_pool.tile([D, TW], RD, tag="yt")
                        nc.vector.tensor_mul(yt[:], yu[0:D, :], bc[:])
                        if DBG and b == 0:
                            nc.sync.dma_start(
                                out=dbgden_d[2 * w + h : 2 * w + h + 1, :],
                                in_=yu[D : D + 1, :],
                            )
                            nc.sync.dma_start(
                                out=dbgrcp_d[2 * w + h : 2 * w + h + 1, :],
                                in_=recip[:],
                            )
                            nc.sync.dma_start(
                                out=dbgy_d[2 * w + h, :, :], in_=yt[:]
                            )
                        # tokens w*512..w*512+511 of batch b go to dests
                        # 2w (first 256) and 2w+1 (second 256)
                        for u in range(2):
                            nc.sync.dma_start(
                                out=y_send[b, 2 * w + u, h * D : (h + 1) * D, :],
                                in_=yt[:, u * TB : (u + 1) * TB],
                            )
                nc.gpsimd.collective_compute(
                    "AllToAll",
                    mybir.AluOpType.bypass,
                    replica_groups=[list(range(NCORES))],
                    ins=[y_send[b, :, :, :].opt()],
                    outs=[y_recv[b, :, :, :].opt()],
                )

            def proj(b):
                """Output projection for this core's 256-token slice of batch b."""
                yr = yr_pool.tile([P, KC, TB], RD, tag="yr")
                nc.sync.dma_start(
                    out=yr[:],
                    in_=y_recv[b, :, :, :].rearrange("k p t -> p k t"),
                )
                if DBG and b == 0:
                    nc.sync.dma_start(out=dbgyr_d[:, :, :], in_=yr[:])
                for jt in range(TB // P):
                    for half in range(C // TW):
                        ps_o = ps1.tile([P, TW], F32, tag="ps1", name="ps_o")
                        for kc in range(KC):
                            nc.tensor.matmul(
                                ps_o[:],
                                lhsT=yr[:, kc, jt * P : (jt + 1) * P],
                                rhs=wproj_sb[:, kc, half * TW : (half + 1) * TW],
                                start=(kc == 0),
                                stop=(kc == KC - 1),
                            )
                        ob = ob_pool.tile([P, TW], F32, tag="ob")
                        nc.vector.tensor_copy(ob[:], ps_o[:])
                        nc.sync.dma_start(
                            out=out_d[
                                b * TB + jt * P : b * TB + (jt + 1) * P,
                                half * TW : (half + 1) * TW,
                            ],
                            in_=ob[:],
                        )

            # ---- schedule: hide each A2A_b + proj(b) under stage1(b+1) ----
            qkv_tiles = stage1(0)
            if DBG:
                nc.sync.dma_start(out=dbgq_d[:, :], in_=qkv_tiles[0][:])
            load_wproj()
            for b in range(B):
                chains(b, qkv_tiles)
                if b + 1 < B:
                    qkv_tiles = stage1(b + 1)
                proj(b)

    nc.finalize()
    return nc


def x_dram_re(handle, pattern):
    return handle[:, :].rearrange(pattern, p=P)


_NC_CACHE: dict = {}


def _get_nc() -> bass.Bass:
    if "nc" not in _NC_CACHE:
        _NC_CACHE["nc"] = build()
    return _NC_CACHE["nc"]


def shard_inputs(x, W_qkv, W_proj):
    x = np.ascontiguousarray(np.asarray(x, dtype=np.float32).reshape(BT, C))
    W_qkv = np.asarray(W_qkv, dtype=np.float32)
    W_proj = np.ascontiguousarray(np.asarray(W_proj, dtype=np.float32))
    in_maps = []
    for c in range(NCORES):
        cols = slice(QKC * c, QKC * (c + 1))
        w_c = np.ascontiguousarray(
            np.concatenate(
                [W_qkv[:, cols], W_qkv[:, C:][:, cols], W_qkv[:, 2 * C :][:, cols]],
                axis=1,
            )
        )
        in_maps.append({"x": x, "wqkv": w_c, "wproj": W_proj})
    return in_maps


def run(in_maps, trace=False, **kwargs):
    return run_bass_kernel_spmd(
        _get_nc(), in_maps, core_ids=list(range(NCORES)), trace=trace, **kwargs
    )


def unshard(results) -> np.ndarray:
    """Core c's out row (b*256 + j) is global token b*2048 + c*256 + j."""
    arr = np.stack([results[c]["out"] for c in range(NCORES)])  # [8, 1024, C]
    return (
        arr.reshape(NCORES, B, TB, C)
        .transpose(1, 0, 2, 3)
        .reshape(B, T, C)
        .astype(np.float32)
    )


def kernel(x, W_qkv, W_proj):
    res = run(shard_inputs(x, W_qkv, W_proj), trace=False)
    return unshard(res.results)


# revision 20
# speedup vs baseline: 1.4651x; 1.0811x over previous
"""Causal self-attention (B=4, T=2048, C=1024, 16 heads) on 8 trn2 NeuronCores.

Sharding: tensor-parallel over heads (2 heads/core) for QKV + attention.
Output tokens are interleaved across cores: core c owns, for every batch b,
the 256-token slice [b*2048 + c*256, b*2048 + (c+1)*256).  This lets the
head-split -> token-split reshard run as FOUR small per-batch AllToAlls
(512KB each), each issued right after its batch's attention finishes and
hidden under the next batch's compute; the output projection for batch b
runs inside the main loop as soon as A2A_b lands.

The host feeds x pre-transposed ([C, BT]) and pre-cast to bf16, and the
per-core W_qkv column slice / W_proj in bf16, so the device does no
layout transposes of x and no f32->bf16 staging at all.

Per-core pipeline (all cores run the identical program; only the fed
W_qkv column-slice differs):
  stage 1: qT,kT [128ch x 2048tok] and v [tok-major] per batch: plain
           matmuls against the DMA'd xT tiles; only v needs a PE
           transpose (4 per window).
  stage 2: causal attention per (batch, head): S^T tiles [kt=128, q<=512],
           software-pipelined with LOOKAHEAD S-matmuls in flight ahead of
           the AV accumulation so the PE never waits on exp/mask.
           Diagonal tiles skip their fully-masked leading columns and
           apply the shared [128,128] lower-triangle mask only to the
           single 128-column block that straddles the diagonal.
           exp on ACT (no max-subtraction: scores/8 ~ N(0,1), bounded),
           ones-column appended to v so PSUM row 64 carries the softmax
           denominators; normalize via reciprocal_approx_fast +
           partition_broadcast.
  stage 3 (per batch): A2A_b -> y^T [1024ch, 256tok] token slice,
           out = y^T.T @ W_proj, interleaved after the next batch's
           stage 1 so the collective latency is hidden.
"""

import os
import numpy as np
import ml_dtypes

from concourse import bass, bacc, mybir, tile
from concourse.bass_utils import run_bass_kernel_spmd

F32 = mybir.dt.float32
BF16 = mybir.dt.bfloat16
RD = BF16

B, T, C = 4, 2048, 1024
H, D = 16, 64
NCORES = 8
HPC = H // NCORES            # heads per core = 2
QKC = HPC * D                # per-core q/k/v channels = 128
BT = B * T                   # 8192 tokens total
TPS = BT // NCORES           # tokens per core in the output = 1024
TB = TPS // B                # tokens per (core, batch) out slice = 256
P = 128
TW = 512                     # token window for stage1/attention q windows
NW = T // TW                 # windows per batch = 4
NKT = T // P                 # kt tiles per batch = 16

# ---- knobs ----
PSS_BUFS = int(os.environ.get("KPSS", "4"))
PSY_BUFS = int(os.environ.get("KPSY", "2"))
LOOKAHEAD = int(os.environ.get("KLA", "3"))
DBG = os.environ.get("KDBG", "0") == "1"


def _full_mask_row() -> np.ndarray:
    """row mask[p, f] = 1.0 iff 128*(NW-1) + p <= f  (the w=0,s=3 tile)."""
    m = np.zeros((P, TW), dtype=ml_dtypes.bfloat16)
    p = np.arange(P)[:, None]
    f = np.arange(TW)[None, :]
    m[(P * (TW // P - 1) + p) <= f] = 1.0
    return m


def _tri_mask() -> np.ndarray:
    """tri[p, f] = 1.0 iff p <= f — shared by every diagonal 128-block."""
    return np.tril(np.ones((P, P), dtype=ml_dtypes.bfloat16)).T


def build() -> bass.Bass:
    nc = bacc.Bacc(num_devices=NCORES, target_bir_lowering=False)

    xt_d = nc.dram_tensor("xt", [C, BT], BF16, kind="ExternalInput")
    wqkv_d = nc.dram_tensor("wqkv", [C, 3 * QKC], BF16, kind="ExternalInput")
    wproj_d = nc.dram_tensor("wproj", [C, C], BF16, kind="ExternalInput")
    out_d = nc.dram_tensor("out", [TPS, C], F32, kind="ExternalOutput")

    maskf_d = nc.inline_tensor(_full_mask_row(), name="maskf")
    tri_d = nc.inline_tensor(_tri_mask(), name="masktri")
    ident_d = nc.inline_tensor(np.eye(P, dtype=ml_dtypes.bfloat16), name="ident")

    KC = C // P  # 8 contraction chunks

    if DBG:
        dbgq_d = nc.dram_tensor("dbg_q", [P, T], RD, kind="ExternalOutput")
        dbgden_d = nc.dram_tensor("dbg_den", [NW * HPC, TW], F32, kind="ExternalOutput")
        dbgrcp_d = nc.dram_tensor("dbg_rcp", [NW * HPC, TW], F32, kind="ExternalOutput")
        dbgy_d = nc.dram_tensor("dbg_y", [NW * HPC, D, TW], RD, kind="ExternalOutput")
        dbgyr_d = nc.dram_tensor("dbg_yr", [P, KC, TB], RD, kind="ExternalOutput")

    with tile.TileContext(nc) as tc:
        from contextlib import ExitStack

        with ExitStack() as ctx:
            # ---- persistent pools ----
            wq_pool = ctx.enter_context(tc.tile_pool(name="wq", bufs=1))
            msk_pool = ctx.enter_context(tc.tile_pool(name="msk", bufs=1))
            wp_pool = ctx.enter_context(tc.tile_pool(name="wp", bufs=1))
            dram = ctx.enter_context(tc.tile_pool(name="dram", bufs=1, space="DRAM"))

            wqkv_sb = wq_pool.tile([P, KC, 3 * QKC], RD)
            for kc in range(KC):
                nc.sync.dma_start(
                    out=wqkv_sb[:, kc, :], in_=wqkv_d[kc * P : (kc + 1) * P, :]
                )
            maskf_sb = msk_pool.tile([P, TW], RD)
            nc.sync.dma_start(out=maskf_sb[:], in_=maskf_d[:, :])
            tri_sb = msk_pool.tile([P, P], RD)
            nc.sync.dma_start(out=tri_sb[:], in_=tri_d[:, :])
            ident_rd = msk_pool.tile([P, P], RD)
            nc.sync.dma_start(out=ident_rd[:], in_=ident_d[:, :])
            wproj_sb = wp_pool.tile([P, KC, C], RD)

            def load_wproj():
                # issued after batch 0's stage 1 so it doesn't steal DMA
                # bandwidth from the startup-critical xT/wqkv loads
                for kc in range(KC):
                    nc.sync.dma_start(
                        out=wproj_sb[:, kc, :],
                        in_=wproj_d[kc * P : (kc + 1) * P, :],
                    )

            y_send = dram.tile([B, NCORES, QKC, TB], RD)
            y_recv = dram.tile([B, NCORES, QKC, TB], RD)

            # ---- stage pools (persistent: stages interleave across batches) ----
            xT_pool = ctx.enter_context(tc.tile_pool(name="xT", bufs=3))
            qkv_pool = ctx.enter_context(tc.tile_pool(name="qkv", bufs=2))
            ps1 = ctx.enter_context(tc.tile_pool(name="ps1", bufs=2, space="PSUM"))
            pss = ctx.enter_context(
                tc.tile_pool(name="pss", bufs=PSS_BUFS, space="PSUM")
            )
            psy = ctx.enter_context(
                tc.tile_pool(name="psy", bufs=PSY_BUFS, space="PSUM")
            )
            pt_pool = ctx.enter_context(tc.tile_pool(name="pt", bufs=6))
            nrm_pool = ctx.enter_context(tc.tile_pool(name="nrm", bufs=2))
            yt_pool = ctx.enter_context(tc.tile_pool(name="yt", bufs=4))
            yr_pool = ctx.enter_context(tc.tile_pool(name="yr", bufs=2))
            ob_pool = ctx.enter_context(tc.tile_pool(name="ob", bufs=2))

            def stage1(b):
                """qT, kT (ch-major) and v (tok-major) for batch b."""
                qT_b = qkv_pool.tile([P, T], RD, tag="qT")
                kT_b = qkv_pool.tile([P, T], RD, tag="kT")
                v_b = qkv_pool.tile([P, NKT, HPC, D + 1], RD, tag="v")
                # ones column for softmax denominators
                nc.gpsimd.memset(v_b[:, :, :, D : D + 1], 1.0)

                for w in range(NW):
                    t0 = b * T + w * TW
                    xTw = xT_pool.tile([P, KC, TW], RD, tag="xT")
                    for kc in range(KC):
                        nc.sync.dma_start(
                            out=xTw[:, kc, :],
                            in_=xt_d[kc * P : (kc + 1) * P, t0 : t0 + TW],
                        )
                    for which, dst in ((0, qT_b), (1, kT_b)):
                        ps = ps1.tile([P, TW], F32, tag="ps1")
                        for kc in range(KC):
                            nc.tensor.matmul(
                                ps[:],
                                lhsT=wqkv_sb[:, kc, which * QKC : (which + 1) * QKC],
                                rhs=xTw[:, kc, :],
                                start=(kc == 0),
                                stop=(kc == KC - 1),
                            )
                        nc.vector.tensor_copy(dst[:, w * TW : (w + 1) * TW], ps[:])
                    ps_vT = ps1.tile([P, TW], F32, tag="ps1", name="ps_vT")
                    for kc in range(KC):
                        nc.tensor.matmul(
                            ps_vT[:],
                            lhsT=wqkv_sb[:, kc, 2 * QKC : 3 * QKC],
                            rhs=xTw[:, kc, :],
                            start=(kc == 0),
                            stop=(kc == KC - 1),
                        )
                    vT_sb = xT_pool.tile([P, TW], RD, tag="vT", name="vT_sb")
                    nc.vector.tensor_copy(vT_sb[:], ps_vT[:])
                    ps_v = ps1.tile([P, TW], RD, tag="ps1", name="ps_v")
                    for s in range(TW // P):
                        nc.tensor.transpose(
                            ps_v[:, s * P : (s + 1) * P],
                            vT_sb[:, s * P : (s + 1) * P],
                            ident_rd[:],
                        )
                    jt0 = w * (TW // P)
                    nc.vector.tensor_copy(
                        v_b[:, jt0 : jt0 + TW // P, :, 0:D],
                        ps_v[:].rearrange("p (s h d) -> p s h d", s=TW // P, h=HPC),
                    )
                return qT_b, kT_b, v_b

            def chains(b, qkv_tiles):
                """Attention chains for batch b, then the per-batch A2A."""
                qT_b, kT_b, v_b = qkv_tiles
                for w in range(NW):
                    for h in range(HPC):
                        qT_h = qT_b[h * D : (h + 1) * D, :]
                        kT_h = kT_b[h * D : (h + 1) * D, :]
                        nkt = (w + 1) * (TW // P)
                        # (jk, col_start): diagonal tiles first; diag tile s
                        # skips its fully-masked first 128*s columns.  The
                        # last item in issue order must be full-width so the
                        # PSUM accumulation group closes over all columns.
                        items = []
                        for s in range(TW // P):
                            cs = P * s
                            if w == 0 and s == TW // P - 1:
                                cs = 0
                            items.append((w * (TW // P) + s, cs))
                        items += [(jk, 0) for jk in range(w * (TW // P))]
                        pts = []

                        def issue_s(ji):
                            jk, cs = items[ji]
                            s = jk - w * (TW // P)
                            ps_s = pss.tile([P, TW], F32, tag="ps_s")
                            nc.tensor.matmul(
                                ps_s[:, cs:],
                                lhsT=kT_h[:, jk * P : (jk + 1) * P],
                                rhs=qT_h[:, w * TW + cs : (w + 1) * TW],
                                start=True,
                                stop=True,
                            )
                            pt = pt_pool.tile([P, TW], RD, tag="pt")
                            nc.scalar.activation(
                                pt[:, cs:],
                                ps_s[:, cs:],
                                mybir.ActivationFunctionType.Exp,
                                scale=1.0 / np.sqrt(D),
                            )
                            if s >= 0:
                                if w == 0 and s == TW // P - 1:
                                    # full-width exception tile: mask all cols
                                    nc.gpsimd.tensor_mul(
                                        pt[:], pt[:], maskf_sb[:]
                                    )
                                else:
                                    # only the 128-col block straddling the
                                    # diagonal needs masking
                                    blk = slice(P * s, P * (s + 1))
                                    nc.gpsimd.tensor_mul(
                                        pt[:, blk], pt[:, blk], tri_sb[:]
                                    )
                            pts.append(pt)

                        for ji in range(min(LOOKAHEAD, nkt)):
                            issue_s(ji)
                        ps_y = psy.tile([D + 1, TW], F32, tag="ps_y")
                        for ji in range(nkt):
                            if ji + LOOKAHEAD < nkt:
                                issue_s(ji + LOOKAHEAD)
                            jk, cs = items[ji]
                            nc.tensor.matmul(
                                ps_y[:, cs:],
                                lhsT=v_b[:, jk, h, :],
                                rhs=pts[ji][:, cs:],
                                start=(ji == 0),
                                stop=(ji == nkt - 1),
                            )
                        yu = yt_pool.tile([D + 1, TW], F32, tag="yu", bufs=4)
                        nc.vector.tensor_copy(yu[:], ps_y[:])
                        # reciprocal_approx_fast misreads partition-offset
                        # inputs; stage the denominator row at partition 0
                        den0 = nrm_pool.tile([1, TW], F32, tag="den0")
                        nc.vector.tensor_copy(den0[:], yu[D : D + 1, :])
                        recip = nrm_pool.tile([1, TW], F32, tag="recip")
                        nc.vector.reciprocal_approx_fast(recip[:], den0[:])
                        bc = nrm_pool.tile([D, TW], F32, tag="bc")
                        nc.gpsimd.partition_broadcast(bc[:], recip[:])
                        yt = yt_pool.tile([D, TW], RD, tag="yt")
                        nc.vector.tensor_mul(yt[:], yu[0:D, :], bc[:])
                        if DBG and b == 0:
                            nc.sync.dma_start(
                                out=dbgden_d[2 * w + h : 2 * w + h + 1, :],
                                in_=yu[D : D + 1, :],
                            )
                            nc.sync.dma_start(
                                out=dbgrcp_d[2 * w + h : 2 * w + h + 1, :],
                                in_=recip[:],
                            )
                            nc.sync.dma_start(
                                out=dbgy_d[2 * w + h, :, :], in_=yt[:]
                            )
                        # tokens w*512..w*512+511 of batch b go to dests
                        # 2w (first 256) and 2w+1 (second 256)
                        for u in range(2):
                            nc.sync.dma_start(
                                out=y_send[b, 2 * w + u, h * D : (h + 1) * D, :],
                                in_=yt[:, u * TB : (u + 1) * TB],
                            )
                nc.gpsimd.collective_compute(
                    "AllToAll",
                    mybir.AluOpType.bypass,
                    replica_groups=[list(range(NCORES))],
                    ins=[y_send[b, :, :, :].opt()],
                    outs=[y_recv[b, :, :, :].opt()],
                )

            def proj(b):
                """Output projection for this core's 256-token slice of batch b."""
                yr = yr_pool.tile([P, KC, TB], RD, tag="yr")
                for kc in range(KC):
                    nc.sync.dma_start(
                        out=yr[:, kc, :], in_=y_recv[b, kc, :, :]
                    )
                if DBG and b == 0:
                    nc.sync.dma_start(out=dbgyr_d[:, :, :], in_=yr[:])
                for jt in range(TB // P):
                    for half in range(C // TW):
                        ps_o = ps1.tile([P, TW], F32, tag="ps1", name="ps_o")
                        for kc in range(KC):
                            nc.tensor.matmul(
                                ps_o[:],
                                lhsT=yr[:, kc, jt * P : (jt + 1) * P],
                                rhs=wproj_sb[:, kc, half * TW : (half + 1) * TW],
                                start=(kc == 0),
                                stop=(kc == KC - 1),
                            )
                        ob = ob_pool.tile([P, TW], F32, tag="ob")
                        nc.vector.tensor_copy(ob[:], ps_o[:])
                        nc.sync.dma_start(
                            out=out_d[
                                b * TB + jt * P : b * TB + (jt + 1) * P,
                                half * TW : (half + 1) * TW,
                            ],
                            in_=ob[:],
                        )

            # ---- schedule: hide each A2A_b + proj(b) under stage1(b+1) ----
            qkv_tiles = stage1(0)
            if DBG:
                nc.sync.dma_start(out=dbgq_d[:, :], in_=qkv_tiles[0][:])
            load_wproj()
            for b in range(B):
                chains(b, qkv_tiles)
                if b + 1 < B:
                    qkv_tiles = stage1(b + 1)
                proj(b)

    nc.finalize()
    return nc


_NC_CACHE: dict = {}


def _get_nc() -> bass.Bass:
    if "nc" not in _NC_CACHE:
        _NC_CACHE["nc"] = build()
    return _NC_CACHE["nc"]


def shard_inputs(x, W_qkv, W_proj):
    x = np.asarray(x, dtype=np.float32).reshape(BT, C)
    xt = np.ascontiguousarray(x.T.astype(ml_dtypes.bfloat16))
    W_qkv = np.asarray(W_qkv, dtype=np.float32)
    wproj = np.ascontiguousarray(
        np.asarray(W_proj, dtype=np.float32).astype(ml_dtypes.bfloat16)
    )
    in_maps = []
    for c in range(NCORES):
        cols = slice(QKC * c, QKC * (c + 1))
        w_c = np.ascontiguousarray(
            np.concatenate(
                [W_qkv[:, cols], W_qkv[:, C:][:, cols], W_qkv[:, 2 * C :][:, cols]],
                axis=1,
            ).astype(ml_dtypes.bfloat16)
        )
        in_maps.append({"xt": xt, "wqkv": w_c, "wproj": wproj})
    return in_maps


def run(in_maps, trace=False, **kwargs):
    return run_bass_kernel_spmd(
        _get_nc(), in_maps, core_ids=list(range(NCORES)), trace=trace, **kwargs
    )


def unshard(results) -> np.ndarray:
    """Core c's out row (b*256 + j) is global token b*2048 + c*256 + j."""
    arr = np.stack([results[c]["out"] for c in range(NCORES)])  # [8, 1024, C]
    return (
        arr.reshape(NCORES, B, TB, C)
        .transpose(1, 0, 2, 3)
        .reshape(B, T, C)
        .astype(np.float32)
    )


def kernel(x, W_qkv, W_proj):
    res = run(shard_inputs(x, W_qkv, W_proj), trace=False)
    return unshard(res.results)


# revision 25
# speedup vs baseline: 2.0276x; 1.3839x over previous
"""Causal self-attention (B=4, T=2048, C=1024, 16 heads) on 8 trn2 NeuronCores.

Sharding: tensor-parallel over heads (2 heads/core) for QKV + attention.
Output tokens are interleaved across cores: core c owns, for every batch b,
the 256-token slice [b*2048 + c*256, b*2048 + (c+1)*256).  This lets the
head-split -> token-split reshard run as FOUR small per-batch AllToAlls
(512KB each), each issued right after its batch's attention finishes and
hidden under the next batch's compute; the output projection for batch b
runs inside the main loop as soon as A2A_b lands.

The host feeds x pre-transposed ([C, BT]) and pre-cast to bf16, and the
per-core W_qkv column slice / W_proj in bf16, so the device does no
layout transposes of x and no f32->bf16 staging at all.

Per-core pipeline (all cores run the identical program; only the fed
W_qkv column-slice differs):
  stage 1: qT,kT [128ch x 2048tok] and v [tok-major] per batch: plain
           matmuls against the DMA'd xT tiles; only v needs a PE
           transpose (4 per window).
  stage 2: causal attention per (batch, head): S^T tiles [kt=128, q<=512],
           software-pipelined with LOOKAHEAD S-matmuls in flight ahead of
           the AV accumulation so the PE never waits on exp/mask.
           Diagonal tiles skip their fully-masked leading columns and
           apply the shared [128,128] lower-triangle mask only to the
           single 128-column block that straddles the diagonal.
           exp on ACT (no max-subtraction: scores/8 ~ N(0,1), bounded),
           ones-column appended to v so PSUM row 64 carries the softmax
           denominators; normalize via reciprocal_approx_fast +
           partition_broadcast.
  stage 3 (per batch): A2A_b -> y^T [1024ch, 256tok] token slice,
           out = y^T.T @ W_proj, interleaved after the next batch's
           stage 1 so the collective latency is hidden.
"""

import os
import numpy as np
import ml_dtypes

from concourse import bass, bacc, mybir, tile
from concourse.bass_utils import run_bass_kernel_spmd

F32 = mybir.dt.float32
BF16 = mybir.dt.bfloat16
RD = BF16

B, T, C = 4, 2048, 1024
H, D = 16, 64
NCORES = 8
HPC = H // NCORES            # heads per core = 2
QKC = HPC * D                # per-core q/k/v channels = 128
BT = B * T                   # 8192 tokens total
TPS = BT // NCORES           # tokens per core in the output = 1024
TB = TPS // B                # tokens per (core, batch) out slice = 256
P = 128
TW = 512                     # token window for stage1/attention q windows
NW = T // TW                 # windows per batch = 4
NKT = T // P                 # kt tiles per batch = 16

# ---- knobs ----
PSS_BUFS = int(os.environ.get("KPSS", "4"))
PSY_BUFS = int(os.environ.get("KPSY", "2"))
LOOKAHEAD = int(os.environ.get("KLA", "4"))
DBG = os.environ.get("KDBG", "0") == "1"


def _full_mask_row() -> np.ndarray:
    """row mask[p, f] = 1.0 iff 128*(NW-1) + p <= f  (the w=0,s=3 tile)."""
    m = np.zeros((P, TW), dtype=ml_dtypes.bfloat16)
    p = np.arange(P)[:, None]
    f = np.arange(TW)[None, :]
    m[(P * (TW // P - 1) + p) <= f] = 1.0
    return m


def _tri_mask() -> np.ndarray:
    """tri[p, f] = 1.0 iff p <= f — shared by every diagonal 128-block."""
    return np.tril(np.ones((P, P), dtype=ml_dtypes.bfloat16)).T


def build() -> bass.Bass:
    nc = bacc.Bacc(num_devices=NCORES, target_bir_lowering=False)

    xt_d = nc.dram_tensor("xt", [C, BT], BF16, kind="ExternalInput")
    wqkv_d = nc.dram_tensor("wqkv", [C, 3 * QKC], BF16, kind="ExternalInput")
    wproj_d = nc.dram_tensor("wproj", [C, C], BF16, kind="ExternalInput")
    out_d = nc.dram_tensor("out", [TPS, C], F32, kind="ExternalOutput")

    maskf_d = nc.inline_tensor(_full_mask_row(), name="maskf")
    tri_d = nc.inline_tensor(_tri_mask(), name="masktri")
    ident_d = nc.inline_tensor(np.eye(P, dtype=ml_dtypes.bfloat16), name="ident")

    KC = C // P  # 8 contraction chunks

    if DBG:
        dbgq_d = nc.dram_tensor("dbg_q", [P, T], RD, kind="ExternalOutput")
        dbgden_d = nc.dram_tensor("dbg_den", [NW * HPC, TW], F32, kind="ExternalOutput")
        dbgrcp_d = nc.dram_tensor("dbg_rcp", [NW * HPC, TW], F32, kind="ExternalOutput")
        dbgy_d = nc.dram_tensor("dbg_y", [NW * HPC, D, TW], RD, kind="ExternalOutput")
        dbgyr_d = nc.dram_tensor("dbg_yr", [P, KC, TB], RD, kind="ExternalOutput")

    with tile.TileContext(nc) as tc:
        from contextlib import ExitStack

        with ExitStack() as ctx:
            # ---- persistent pools ----
            wq_pool = ctx.enter_context(tc.tile_pool(name="wq", bufs=1))
            msk_pool = ctx.enter_context(tc.tile_pool(name="msk", bufs=1))
            wp_pool = ctx.enter_context(tc.tile_pool(name="wp", bufs=1))
            dram = ctx.enter_context(tc.tile_pool(name="dram", bufs=1, space="DRAM"))

            wqkv_sb = wq_pool.tile([P, KC, 3 * QKC], RD)
            for kc in range(KC):
                nc.sync.dma_start(
                    out=wqkv_sb[:, kc, :], in_=wqkv_d[kc * P : (kc + 1) * P, :]
                )
            maskf_sb = msk_pool.tile([P, TW], RD)
            nc.sync.dma_start(out=maskf_sb[:], in_=maskf_d[:, :])
            tri_sb = msk_pool.tile([P, P], RD)
            nc.sync.dma_start(out=tri_sb[:], in_=tri_d[:, :])
            ident_rd = msk_pool.tile([P, P], RD)
            nc.sync.dma_start(out=ident_rd[:], in_=ident_d[:, :])
            wproj_sb = wp_pool.tile([P, KC, C], RD)

            def load_wproj():
                # issued after batch 0's stage 1 so it doesn't steal DMA
                # bandwidth from the startup-critical xT/wqkv loads
                for kc in range(KC):
                    nc.sync.dma_start(
                        out=wproj_sb[:, kc, :],
                        in_=wproj_d[kc * P : (kc + 1) * P, :],
                    )

            y_send = dram.tile([B, NCORES, QKC, TB], RD)
            y_recv = dram.tile([B, NCORES, QKC, TB], RD)

            # ---- stage pools (persistent: stages interleave across batches) ----
            xT_pool = ctx.enter_context(tc.tile_pool(name="xT", bufs=3))
            qkv_pool = ctx.enter_context(tc.tile_pool(name="qkv", bufs=2))
            ps1 = ctx.enter_context(tc.tile_pool(name="ps1", bufs=2, space="PSUM"))
            pss = ctx.enter_context(
                tc.tile_pool(name="pss", bufs=PSS_BUFS, space="PSUM")
            )
            psy = ctx.enter_context(
                tc.tile_pool(name="psy", bufs=PSY_BUFS, space="PSUM")
            )
            pt_pool = ctx.enter_context(tc.tile_pool(name="pt", bufs=6))
            nrm_pool = ctx.enter_context(tc.tile_pool(name="nrm", bufs=2))
            yt_pool = ctx.enter_context(tc.tile_pool(name="yt", bufs=4))
            yr_pool = ctx.enter_context(tc.tile_pool(name="yr", bufs=2))
            ob_pool = ctx.enter_context(tc.tile_pool(name="ob", bufs=2))

            def stage1(b):
                """qT, kT (ch-major) and v (tok-major) for batch b."""
                qT_b = qkv_pool.tile([P, T], RD, tag="qT")
                kT_b = qkv_pool.tile([P, T], RD, tag="kT")
                v_b = qkv_pool.tile([P, NKT, HPC, D + 1], RD, tag="v")
                # ones column for softmax denominators (on Vector: the GpSimd
                # queue must stay clear of everything the chains depend on,
                # since the collectives block it for ~10us at batch bounds)
                nc.vector.memset(v_b[:, :, :, D : D + 1], 1.0)

                for w in range(NW):
                    t0 = b * T + w * TW
                    xTw = xT_pool.tile([P, KC, TW], RD, tag="xT")
                    for kc in range(KC):
                        nc.sync.dma_start(
                            out=xTw[:, kc, :],
                            in_=xt_d[kc * P : (kc + 1) * P, t0 : t0 + TW],
                        )
                    for which, dst in ((0, qT_b), (1, kT_b)):
                        ps = ps1.tile([P, TW], F32, tag="ps1")
                        for kc in range(KC):
                            nc.tensor.matmul(
                                ps[:],
                                lhsT=wqkv_sb[:, kc, which * QKC : (which + 1) * QKC],
                                rhs=xTw[:, kc, :],
                                start=(kc == 0),
                                stop=(kc == KC - 1),
                            )
                        nc.vector.tensor_copy(dst[:, w * TW : (w + 1) * TW], ps[:])
                    ps_vT = ps1.tile([P, TW], F32, tag="ps1", name="ps_vT")
                    for kc in range(KC):
                        nc.tensor.matmul(
                            ps_vT[:],
                            lhsT=wqkv_sb[:, kc, 2 * QKC : 3 * QKC],
                            rhs=xTw[:, kc, :],
                            start=(kc == 0),
                            stop=(kc == KC - 1),
                        )
                    vT_sb = xT_pool.tile([P, TW], RD, tag="vT", name="vT_sb")
                    nc.vector.tensor_copy(vT_sb[:], ps_vT[:])
                    ps_v = ps1.tile([P, TW], RD, tag="ps1", name="ps_v")
                    for s in range(TW // P):
                        nc.tensor.transpose(
                            ps_v[:, s * P : (s + 1) * P],
                            vT_sb[:, s * P : (s + 1) * P],
                            ident_rd[:],
                        )
                    jt0 = w * (TW // P)
                    nc.vector.tensor_copy(
                        v_b[:, jt0 : jt0 + TW // P, :, 0:D],
                        ps_v[:].rearrange("p (s h d) -> p s h d", s=TW // P, h=HPC),
                    )
                return qT_b, kT_b, v_b

            def chains(b, qkv_tiles):
                """Attention chains for batch b, then the per-batch A2A."""
                qT_b, kT_b, v_b = qkv_tiles
                tails = []
                for w in range(NW):
                    for h in range(HPC):
                        qT_h = qT_b[h * D : (h + 1) * D, :]
                        kT_h = kT_b[h * D : (h + 1) * D, :]
                        nkt = (w + 1) * (TW // P)
                        # (jk, col_start): diagonal tiles first; diag tile s
                        # skips its fully-masked first 128*s columns.  The
                        # last item in issue order must be full-width so the
                        # PSUM accumulation group closes over all columns.
                        items = []
                        for s in range(TW // P):
                            cs = P * s
                            if w == 0 and s == TW // P - 1:
                                cs = 0
                            items.append((w * (TW // P) + s, cs))
                        items += [(jk, 0) for jk in range(w * (TW // P))]
                        pts = []

                        def issue_s(ji):
                            jk, cs = items[ji]
                            s = jk - w * (TW // P)
                            ps_s = pss.tile([P, TW], F32, tag="ps_s")
                            nc.tensor.matmul(
                                ps_s[:, cs:],
                                lhsT=kT_h[:, jk * P : (jk + 1) * P],
                                rhs=qT_h[:, w * TW + cs : (w + 1) * TW],
                                start=True,
                                stop=True,
                            )
                            pt = pt_pool.tile([P, TW], RD, tag="pt")
                            nc.scalar.activation(
                                pt[:, cs:],
                                ps_s[:, cs:],
                                mybir.ActivationFunctionType.Exp,
                                scale=1.0 / np.sqrt(D),
                            )
                            if s >= 0:
                                if w == 0 and s == TW // P - 1:
                                    # full-width exception tile: mask all cols
                                    nc.vector.tensor_mul(
                                        pt[:], pt[:], maskf_sb[:]
                                    )
                                else:
                                    # only the 128-col block straddling the
                                    # diagonal needs masking
                                    blk = slice(P * s, P * (s + 1))
                                    nc.vector.tensor_mul(
                                        pt[:, blk], pt[:, blk], tri_sb[:]
                                    )
                            pts.append(pt)

                        for ji in range(min(LOOKAHEAD, nkt)):
                            issue_s(ji)
                        ps_y = psy.tile([D + 1, TW], F32, tag="ps_y")
                        for ji in range(nkt):
                            if ji + LOOKAHEAD < nkt:
                                issue_s(ji + LOOKAHEAD)
                            jk, cs = items[ji]
                            nc.tensor.matmul(
                                ps_y[:, cs:],
                                lhsT=v_b[:, jk, h, :],
                                rhs=pts[ji][:, cs:],
                                start=(ji == 0),
                                stop=(ji == nkt - 1),
                            )
                        yu = yt_pool.tile([D + 1, TW], F32, tag="yu", bufs=8)
                        nc.vector.tensor_copy(yu[:], ps_y[:])
                        # reciprocal_approx_fast misreads partition-offset
                        # inputs; stage the denominator row at partition 0
                        den0 = nrm_pool.tile([1, TW], F32, tag="den0")
                        nc.vector.tensor_copy(den0[:], yu[D : D + 1, :])
                        recip = nrm_pool.tile([1, TW], F32, tag="recip", bufs=8)
                        nc.vector.reciprocal_approx_fast(recip[:], den0[:])
                        tails.append((w, h, yu, recip))
                # batch-end normalize + sends: the GpSimd broadcasts run as
                # one contiguous block while the collective queue is idle, so
                # neither the broadcasts nor the vector multiplies that wait
                # on them ever gate the next batch's chain-start masks
                for w, h, yu, recip in tails:
                    bc = nrm_pool.tile([D, TW], F32, tag="bc")
                    nc.gpsimd.partition_broadcast(bc[:], recip[:])
                    yt = yt_pool.tile([D, TW], RD, tag="yt")
                    nc.vector.tensor_mul(yt[:], yu[0:D, :], bc[:])
                    if DBG and b == 0:
                        nc.sync.dma_start(
                            out=dbgden_d[2 * w + h : 2 * w + h + 1, :],
                            in_=yu[D : D + 1, :],
                        )
                        nc.sync.dma_start(
                            out=dbgrcp_d[2 * w + h : 2 * w + h + 1, :],
                            in_=recip[:],
                        )
                        nc.sync.dma_start(
                            out=dbgy_d[2 * w + h, :, :], in_=yt[:]
                        )
                    # tokens w*512..w*512+511 of batch b go to dests
                    # 2w (first 256) and 2w+1 (second 256)
                    for u in range(2):
                        nc.sync.dma_start(
                            out=y_send[b, 2 * w + u, h * D : (h + 1) * D, :],
                            in_=yt[:, u * TB : (u + 1) * TB],
                        )
                nc.gpsimd.collective_compute(
                    "AllToAll",
                    mybir.AluOpType.bypass,
                    replica_groups=[list(range(NCORES))],
                    ins=[y_send[b, :, :, :].opt()],
                    outs=[y_recv[b, :, :, :].opt()],
                )

            def proj(b):
                """Output projection for this core's 256-token slice of batch b."""
                yr = yr_pool.tile([P, KC, TB], RD, tag="yr")
                for kc in range(KC):
                    nc.sync.dma_start(
                        out=yr[:, kc, :], in_=y_recv[b, kc, :, :]
                    )
                if DBG and b == 0:
                    nc.sync.dma_start(out=dbgyr_d[:, :, :], in_=yr[:])
                for jt in range(TB // P):
                    for half in range(C // TW):
                        ps_o = ps1.tile([P, TW], F32, tag="ps1", name="ps_o")
                        for kc in range(KC):
                            nc.tensor.matmul(
                                ps_o[:],
                                lhsT=yr[:, kc, jt * P : (jt + 1) * P],
                                rhs=wproj_sb[:, kc, half * TW : (half + 1) * TW],
                                start=(kc == 0),
                                stop=(kc == KC - 1),
                            )
                        ob = ob_pool.tile([P, TW], F32, tag="ob")
                        nc.vector.tensor_copy(ob[:], ps_o[:])
                        nc.sync.dma_start(
                            out=out_d[
                                b * TB + jt * P : b * TB + (jt + 1) * P,
                                half * TW : (half + 1) * TW,
                            ],
                            in_=ob[:],
                        )

            # ---- schedule: hide each A2A_b + proj(b) under stage1(b+1) ----
            qkv_tiles = stage1(0)
            if DBG:
                nc.sync.dma_start(out=dbgq_d[:, :], in_=qkv_tiles[0][:])
            load_wproj()
            for b in range(B):
                chains(b, qkv_tiles)
                if b + 1 < B:
                    qkv_tiles = stage1(b + 1)
                proj(b)

    nc.finalize()
    return nc


_NC_CACHE: dict = {}


def _get_nc() -> bass.Bass:
    if "nc" not in _NC_CACHE:
        _NC_CACHE["nc"] = build()
    return _NC_CACHE["nc"]


def shard_inputs(x, W_qkv, W_proj):
    x = np.asarray(x, dtype=np.float32).reshape(BT, C)
    xt = np.ascontiguousarray(x.T.astype(ml_dtypes.bfloat16))
    W_qkv = np.asarray(W_qkv, dtype=np.float32)
    wproj = np.ascontiguousarray(
        np.asarray(W_proj, dtype=np.float32).astype(ml_dtypes.bfloat16)
    )
    in_maps = []
    for c in range(NCORES):
        cols = slice(QKC * c, QKC * (c + 1))
        w_c = np.ascontiguousarray(
            np.concatenate(
                [W_qkv[:, cols], W_qkv[:, C:][:, cols], W_qkv[:, 2 * C :][:, cols]],
                axis=1,
            ).astype(ml_dtypes.bfloat16)
        )
        in_maps.append({"xt": xt, "wqkv": w_c, "wproj": wproj})
    return in_maps


def run(in_maps, trace=False, **kwargs):
    return run_bass_kernel_spmd(
        _get_nc(), in_maps, core_ids=list(range(NCORES)), trace=trace, **kwargs
    )


def unshard(results) -> np.ndarray:
    """Core c's out row (b*256 + j) is global token b*2048 + c*256 + j."""
    arr = np.stack([results[c]["out"] for c in range(NCORES)])  # [8, 1024, C]
    return (
        arr.reshape(NCORES, B, TB, C)
        .transpose(1, 0, 2, 3)
        .reshape(B, T, C)
        .astype(np.float32)
    )


def kernel(x, W_qkv, W_proj):
    res = run(shard_inputs(x, W_qkv, W_proj), trace=False)
    return unshard(res.results)
